# revision 4
# baseline (speedup 1.0000x reference)
"""Trainium2 Bass kernel for nn_ProjectedLinear.

Reference computation:
    out = x @ A @ B @ W_small.T @ B.T @ A.T        x:[4,2048,4096]

Algebraic restructuring: C = B @ W_small.T @ B.T is only [64,64], so
    out = ((x @ (A @ C)) @ A.T)
which cuts the FLOPs to two thin matmuls per token (contract 4096 -> 64,
then 64 -> 4096) and never touches W_small in the main loop.

Distribution: data-parallel over the 8 NeuronCores — each core gets
1024 rows of x (flattened [8192, 4096]) and produces 1024 rows of out.
A/B/W_small are replicated; every core redundantly computes C, A^T and
A2 = A @ C on device (a few hundred small PE ops, hidden under the
initial x DMA).

Per-core dataflow (f32r = tf32-like reduced-precision fp32 matmul mode,
1 cycle/row on the PE vs 4 for exact fp32):
  - PE-transposes x tiles (exact, fp32) k-tile by k-tile,
  - u^T[64,512] += A2[k].T @ x^T[k]   accumulated in PSUM over 32 k-tiles,
  - out[128,512] = (u^T slice).T @ A^T slice  per (m-sub, d) tile,
  - PSUM -> SBUF copies on the DVE, 2 MiB DMA loads/stores on the two
    HWDGE rings.
"""

from contextlib import ExitStack

import numpy as np

import concourse.bass as bass
import concourse.mybir as mybir
import concourse.tile as tile
from concourse import bass_utils
from concourse.masks import make_identity

F32 = mybir.dt.float32
F32R = mybir.dt.float32r

N_CORES = 8
BB, SS, DL, RK, DS = 4, 2048, 4096, 64, 768
M_TOT = BB * SS              # 8192 rows
M_CORE = M_TOT // N_CORES    # 1024 rows per core
MACRO = 512                  # rows per macro-tile (mm moving dim)
N_MACRO = M_CORE // MACRO    # 2
N_SUB = MACRO // 128         # 4 m-subtiles per macro
KT = DL // 128               # 32 k-tiles
ND = DL // 512               # 8 d-slices of 512 for the output matmul

_wsplit_counter = [0]


def _split_multi_waits(nc) -> int:
    """This container's walrus build rejects >1 sync-wait per instruction
    (one wait slot in the TRN instruction encoding). Split any
    multi-wait instruction into single-wait NoOps on the same engine
    queue directly before it — queue order makes this equivalent."""
    n_split = 0
    for fn in nc.m.functions:
        for blk in fn.blocks:
            out = []
            for inst in blk.instructions:
                si = inst.sync_info
                if si is not None and si.on_wait and len(si.on_wait) > 1:
                    waits = list(si.on_wait)
                    for w in waits[:-1]:
                        _wsplit_counter[0] += 1
                        out.append(
                            mybir.InstNoOp(
                                name=f"wsplit-{_wsplit_counter[0]}",
                                engine=inst.engine,
                                sync_info=mybir.SyncInfo(on_wait=[w], on_update=[]),
                            )
                        )
                    si.on_wait = [waits[-1]]
                    n_split += 1
                out.append(inst)
            if len(out) != len(blk.instructions):
                blk.instructions[:] = out
    return n_split


def build_nc() -> bass.Bass:
    nc = bass.Bass("TRN2", target_bir_lowering=False, debug=False, num_devices=1)

    x_d = nc.dram_tensor("x", [M_CORE, DL], F32, kind="ExternalInput").ap()
    a_d = nc.dram_tensor("A", [DL, RK], F32, kind="ExternalInput").ap()
    b_d = nc.dram_tensor("B", [RK, DS], F32, kind="ExternalInput").ap()
    w_d = nc.dram_tensor("W_small", [DS, DS], F32, kind="ExternalInput").ap()
    o_d = nc.dram_tensor("out", [M_CORE, DL], F32, kind="ExternalOutput").ap()

    with tile.TileContext(nc) as tc, ExitStack() as ctx:
        const = ctx.enter_context(tc.tile_pool(name="const", bufs=1))
        persist = ctx.enter_context(tc.tile_pool(name="persist", bufs=1))

        ident = const.tile([128, 128], F32)
        make_identity(nc, ident[:])

        # Persistent weights (f32r): A^T [64, 4096] and A2 = A @ C [128, 32*64]
        at_t = persist.tile([64, DL], F32R)
        a2_t = persist.tile([128, KT * RK], F32R)

        # ---------------- setup: C = B @ W^T @ B^T, A^T, A2 = A @ C --------
        with (
            tc.tile_pool(name="setup", bufs=1) as sp,
            tc.tile_pool(name="ps_setup", bufs=1, space="PSUM") as pss,
        ):
            NJ = DS // 128  # 6 tiles over the 768 dim
            b_f32 = sp.tile([RK, DS], F32)
            nc.sync.dma_start(b_f32[:], b_d[:])
            a_f32 = sp.tile([128, KT * RK], F32)
            nc.sync.dma_start(
                a_f32[:].rearrange("p (k r) -> p k r", k=KT),
                a_d.rearrange("(k p) r -> p k r", p=128),
            )
            w_tiles = []
            for j in range(NJ):
                wt = sp.tile([128, DS], F32R, tag=f"wt{j}")
                nc.sync.dma_start(wt[:], w_d[j * 128 : (j + 1) * 128, :].bitcast(F32R))
                w_tiles.append(wt)

            # B^T tiles [128, 64] via PE transpose (exact f32, cast on copy)
            bt_tiles = []
            for j in range(NJ):
                ps = pss.tile([128, RK], F32, tag="ps_bt")
                nc.tensor.transpose(
                    ps[:], b_f32[:, j * 128 : (j + 1) * 128], ident[0:RK, 0:RK]
                )
                bt = sp.tile([128, RK], F32R, tag=f"bt{j}")
                nc.vector.tensor_copy(bt[:], ps[:])
                bt_tiles.append(bt)

            # D[c] = sum_j W[j,c-slice].T @ B^T[j]   -> [128, 64] per c-tile
            d_tiles = []
            for c in range(NJ):
                ps = pss.tile([128, RK], F32, tag="ps_d")
                for j in range(NJ):
                    nc.tensor.matmul(
                        ps[:],
                        w_tiles[j][:, c * 128 : (c + 1) * 128],
                        bt_tiles[j][:],
                        start=(j == 0),
                        stop=(j == NJ - 1),
                    )
                dt = sp.tile([128, RK], F32R, tag=f"d{c}")
                nc.vector.tensor_copy(dt[:], ps[:])
                d_tiles.append(dt)

            # C = sum_c B^T[c].T @ D[c]  -> [64, 64]
            ps_c = pss.tile([RK, RK], F32, tag="ps_c")
            for c in range(NJ):
                nc.tensor.matmul(
                    ps_c[:],
                    bt_tiles[c][:],
                    d_tiles[c][:],
                    start=(c == 0),
                    stop=(c == NJ - 1),
                )
            c_t = sp.tile([RK, RK], F32R)
            nc.vector.tensor_copy(c_t[:], ps_c[:])

            # A^T via PE transposes, 4 k-tiles per PSUM bank
            for g in range(KT // 4):
                ps = pss.tile([RK, 512], F32, tag="ps_at")
                for kk in range(4):
                    k = g * 4 + kk
                    nc.tensor.transpose(
                        ps[:, kk * 128 : (kk + 1) * 128],
                        a_f32[:, k * RK : (k + 1) * RK],
                        ident[:],
                    )
                nc.vector.tensor_copy(at_t[:, g * 512 : (g + 1) * 512], ps[:])

            # A2[k] = A[k-tile] @ C = (A^T slice).T @ C  -> [128, 64]
            for g in range(KT // 8):
                ps = pss.tile([128, 512], F32, tag="ps_a2")
                for kk in range(8):
                    k = g * 8 + kk
                    nc.tensor.matmul(
                        ps[:, kk * RK : (kk + 1) * RK],
                        at_t[:, k * 128 : (k + 1) * 128],
                        c_t[:],
                        start=True,
                        stop=True,
                    )
                nc.vector.tensor_copy(a2_t[:, g * 512 : (g + 1) * 512], ps[:])

        # ---------------- main loop ----------------------------------------
        with (
            tc.tile_pool(name="xn", bufs=6) as xn_pool,
            tc.tile_pool(name="xt", bufs=4) as xt_pool,
            tc.tile_pool(name="ut", bufs=2) as ut_pool,
            tc.tile_pool(name="osb", bufs=3) as out_pool,
            tc.tile_pool(name="ps_t", bufs=2, space="PSUM") as pst,
            tc.tile_pool(name="ps_u", bufs=2, space="PSUM") as psu,
            tc.tile_pool(name="ps_o", bufs=3, space="PSUM") as pso,
        ):
            for mb in range(N_MACRO):
                m0 = mb * MACRO
                xn = []
                for s in range(N_SUB):
                    t = xn_pool.tile([128, DL], F32, tag="xn")
                    r0 = m0 + s * 128
                    nc.sync.dma_start(t[:], x_d[r0 : r0 + 128, :])
                    xn.append(t)

                u_ps = psu.tile([RK, MACRO], F32, tag="u")
                for k in range(KT):
                    t_ps = pst.tile([128, MACRO], F32, tag="t")
                    for s in range(N_SUB):
                        nc.tensor.transpose(
                            t_ps[:, s * 128 : (s + 1) * 128],
                            xn[s][:, k * 128 : (k + 1) * 128],
                            ident[:],
                        )
                    xt = xt_pool.tile([128, MACRO], F32R, tag="xt")
                    nc.vector.tensor_copy(xt[:], t_ps[:])
                    nc.tensor.matmul(
                        u_ps[:],
                        a2_t[:, k * RK : (k + 1) * RK],
                        xt[:],
                        start=(k == 0),
                        stop=(k == KT - 1),
                    )
                u_t = ut_pool.tile([RK, MACRO], F32R, tag="ut")
                nc.vector.tensor_copy(u_t[:], u_ps[:])

                for s in range(N_SUB):
                    o_sb = out_pool.tile([128, DL], F32, tag="osb")
                    for d in range(ND):
                        o_ps = pso.tile([128, 512], F32, tag="o")
                        nc.tensor.matmul(
                            o_ps[:],
                            u_t[:, s * 128 : (s + 1) * 128],
                            at_t[:, d * 512 : (d + 1) * 512],
                            start=True,
                            stop=True,
                        )
                        nc.vector.tensor_copy(o_sb[:, d * 512 : (d + 1) * 512], o_ps[:])
                    r0 = m0 + s * 128
                    nc.scalar.dma_start(o_d[r0 : r0 + 128, :], o_sb[:])

    _split_multi_waits(nc)
    return nc


_NC_CACHE = None


def _get_nc() -> bass.Bass:
    global _NC_CACHE
    if _NC_CACHE is None:
        _NC_CACHE = build_nc()
    return _NC_CACHE


def kernel(x, A, B, W_small) -> np.ndarray:
    x = np.ascontiguousarray(np.asarray(x, dtype=np.float32))
    A = np.ascontiguousarray(np.asarray(A, dtype=np.float32))
    B = np.ascontiguousarray(np.asarray(B, dtype=np.float32))
    W_small = np.ascontiguousarray(np.asarray(W_small, dtype=np.float32))

    xf = x.reshape(M_TOT, DL)
    in_maps = [
        {
            "x": xf[c * M_CORE : (c + 1) * M_CORE],
            "A": A,
            "B": B,
            "W_small": W_small,
        }
        for c in range(N_CORES)
    ]
    nc = _get_nc()
    res = bass_utils.run_bass_kernel_spmd(nc, in_maps, core_ids=list(range(N_CORES)))
    out = np.concatenate([res.results[c]["out"] for c in range(N_CORES)], axis=0)
    return out.reshape(BB, SS, DL)


# revision 5
# speedup vs baseline: 1.1099x; 1.1099x over previous
"""Trainium2 Bass kernel for nn_ProjectedLinear.

Reference computation:
    out = x @ A @ B @ W_small.T @ B.T @ A.T        x:[4,2048,4096]

Algebraic restructuring: C = B @ W_small.T @ B.T is only [64,64], so
    out = (x @ A) @ (C @ A.T)
which cuts the FLOPs to two thin matmuls per token (contract 4096 -> 64,
then 64 -> 4096) and never touches W_small in the main loop.

Distribution: data-parallel over the 8 NeuronCores — each core gets
1024 rows of x (flattened [8192, 4096]) and produces 1024 rows of out.
A/B/W_small are replicated; every core redundantly computes C and
CA^T = C @ A^T on device (small PE ops, hidden under the x DMA).

Per-core dataflow (f32r = tf32-like reduced-precision fp32 matmul mode,
1 cycle/row on the PE vs 4 for exact fp32):
  - x streams in as [128, 512] chunks on the SP HWDGE ring,
  - PE-transposes x k-slices (exact fp32),
  - v^T[64,512] += A[k].T @ x^T[k]     accumulated in PSUM over 32 k-tiles
    (stationary = natural A k-tiles straight from DRAM),
  - out[128,1024] = (v^T slice).T @ CA^T slice   per (m-sub, d) pair,
  - PSUM -> SBUF copies on the DVE; weights + 2 MiB output stores ride
    the ACT HWDGE ring so they never queue behind x loads.
"""

from contextlib import ExitStack

import numpy as np

import concourse.bass as bass
import concourse.mybir as mybir
import concourse.tile as tile
from concourse import bass_utils
from concourse.masks import make_identity

F32 = mybir.dt.float32
F32R = mybir.dt.float32r

N_CORES = 8
BB, SS, DL, RK, DS = 4, 2048, 4096, 64, 768
M_TOT = BB * SS              # 8192 rows
M_CORE = M_TOT // N_CORES    # 1024 rows per core
MACRO = 512                  # rows per macro-tile (mm1 moving dim)
N_MACRO = M_CORE // MACRO    # 2
N_SUB = MACRO // 128         # 4 m-subtiles per macro
KT = DL // 128               # 32 k-tiles
KC = 4                       # k-tiles per x chunk ([128, 512] chunks)
ND = DL // 1024              # 4 d-pairs of 1024 for the output matmul

_wsplit_counter = [0]


def _split_multi_waits(nc) -> int:
    """This container's walrus build rejects >1 sync-wait per instruction
    (one wait slot in the TRN instruction encoding). Split any
    multi-wait instruction into single-wait NoOps on the same engine
    queue directly before it — queue order makes this equivalent."""
    n_split = 0
    for fn in nc.m.functions:
        for blk in fn.blocks:
            out = []
            for inst in blk.instructions:
                si = inst.sync_info
                if si is not None and si.on_wait and len(si.on_wait) > 1:
                    waits = list(si.on_wait)
                    for w in waits[:-1]:
                        _wsplit_counter[0] += 1
                        out.append(
                            mybir.InstNoOp(
                                name=f"wsplit-{_wsplit_counter[0]}",
                                engine=inst.engine,
                                sync_info=mybir.SyncInfo(on_wait=[w], on_update=[]),
                            )
                        )
                    si.on_wait = [waits[-1]]
                    n_split += 1
                out.append(inst)
            if len(out) != len(blk.instructions):
                blk.instructions[:] = out
    return n_split


def build_nc() -> bass.Bass:
    nc = bass.Bass("TRN2", target_bir_lowering=False, debug=False, num_devices=1)

    x_d = nc.dram_tensor("x", [M_CORE, DL], F32, kind="ExternalInput").ap()
    a_d = nc.dram_tensor("A", [DL, RK], F32, kind="ExternalInput").ap()
    b_d = nc.dram_tensor("B", [RK, DS], F32, kind="ExternalInput").ap()
    w_d = nc.dram_tensor("W_small", [DS, DS], F32, kind="ExternalInput").ap()
    o_d = nc.dram_tensor("out", [M_CORE, DL], F32, kind="ExternalOutput").ap()

    with tile.TileContext(nc) as tc, ExitStack() as ctx:
        const = ctx.enter_context(tc.tile_pool(name="const", bufs=1))
        persist = ctx.enter_context(tc.tile_pool(name="persist", bufs=1))

        ident = const.tile([128, 128], F32)
        make_identity(nc, ident[:])

        # Persistent weights: A natural (f32r, mm1 stationary), both as
        # [128, 32*64] (column k holds A[k*128:(k+1)*128, :]), and
        # CA^T = C @ A^T as [64, 4096] (f32r, mm2 moving operand).
        a_nat = persist.tile([128, KT * RK], F32R)
        cat_t = persist.tile([RK, DL], F32R)
        nc.sync.dma_start(
            a_nat[:].rearrange("p (k r) -> p k r", k=KT),
            a_d.rearrange("(k p) r -> p k r", p=128).bitcast(F32R),
        )

        # ------------- setup: C = B @ W^T @ B^T, A^T, CA^T ------------------
        with (
            tc.tile_pool(name="setup", bufs=1) as sp,
            tc.tile_pool(name="ps_setup", bufs=1, space="PSUM") as pss,
        ):
            NJ = DS // 128  # 6 tiles over the 768 dim
            b_f32 = sp.tile([RK, DS], F32)
            nc.scalar.dma_start(b_f32[:], b_d[:])
            w_tiles = []
            for j in range(NJ):
                wt = sp.tile([128, DS], F32R, tag=f"wt{j}")
                nc.scalar.dma_start(
                    wt[:], w_d[j * 128 : (j + 1) * 128, :].bitcast(F32R)
                )
                w_tiles.append(wt)

            # B^T tiles [128, 64] via PE transpose (exact f32, cast on copy)
            bt_tiles = []
            for j in range(NJ):
                ps = pss.tile([128, RK], F32, tag="ps_bt")
                nc.tensor.transpose(
                    ps[:], b_f32[:, j * 128 : (j + 1) * 128], ident[0:RK, 0:RK]
                )
                bt = sp.tile([128, RK], F32R, tag=f"bt{j}")
                nc.vector.tensor_copy(bt[:], ps[:])
                bt_tiles.append(bt)

            # D[c] = sum_j W[j,c-slice].T @ B^T[j]   -> [128, 64] per c-tile
            d_tiles = []
            for c in range(NJ):
                ps = pss.tile([128, RK], F32, tag="ps_d")
                for j in range(NJ):
                    nc.tensor.matmul(
                        ps[:],
                        w_tiles[j][:, c * 128 : (c + 1) * 128],
                        bt_tiles[j][:],
                        start=(j == 0),
                        stop=(j == NJ - 1),
                    )
                dt = sp.tile([128, RK], F32R, tag=f"d{c}")
                nc.vector.tensor_copy(dt[:], ps[:])
                d_tiles.append(dt)

            # C = sum_c B^T[c].T @ D[c]  -> [64, 64], keep C^T for CA^T
            ps_c = pss.tile([RK, RK], F32, tag="ps_c")
            for c in range(NJ):
                nc.tensor.matmul(
                    ps_c[:],
                    bt_tiles[c][:],
                    d_tiles[c][:],
                    start=(c == 0),
                    stop=(c == NJ - 1),
                )
            c_f32 = sp.tile([RK, RK], F32)
            nc.vector.tensor_copy(c_f32[:], ps_c[:])
            ps_ct = pss.tile([RK, RK], F32, tag="ps_ct")
            nc.tensor.transpose(ps_ct[:], c_f32[:], ident[0:RK, 0:RK])
            ct_t = sp.tile([RK, RK], F32R)
            nc.vector.tensor_copy(ct_t[:], ps_ct[:])

            # A^T via PE transposes, 4 k-tiles per PSUM bank
            at_t = sp.tile([RK, DL], F32R)
            for g in range(KT // 4):
                ps = pss.tile([RK, 512], F32, tag="ps_at")
                for kk in range(4):
                    k = g * 4 + kk
                    nc.tensor.transpose(
                        ps[:, kk * 128 : (kk + 1) * 128],
                        a_nat[:, k * RK : (k + 1) * RK].bitcast(F32),
                        ident[:],
                    )
                nc.vector.tensor_copy(at_t[:, g * 512 : (g + 1) * 512], ps[:])

            # CA^T = C @ A^T = (C^T).T @ A^T  -> [64, 4096]
            for g in range(DL // 512):
                ps = pss.tile([RK, 512], F32, tag="ps_cat")
                nc.tensor.matmul(
                    ps[:],
                    ct_t[:],
                    at_t[:, g * 512 : (g + 1) * 512],
                    start=True,
                    stop=True,
                )
                nc.vector.tensor_copy(cat_t[:, g * 512 : (g + 1) * 512], ps[:])

        # ---------------- main loop ----------------------------------------
        with (
            tc.tile_pool(name="xc", bufs=20) as xc_pool,
            tc.tile_pool(name="xt", bufs=8) as xt_pool,
            tc.tile_pool(name="vt", bufs=2) as vt_pool,
            tc.tile_pool(name="osb", bufs=4) as out_pool,
            tc.tile_pool(name="ps_t", bufs=2, space="PSUM") as pst,
            tc.tile_pool(name="ps_v", bufs=2, space="PSUM") as psv,
            tc.tile_pool(name="ps_o", bufs=2, space="PSUM") as pso,
        ):
            for mb in range(N_MACRO):
                m0 = mb * MACRO
                v_ps = psv.tile([RK, MACRO], F32, tag="v")
                for cg in range(KT // KC):
                    # x chunks for k-tiles [cg*KC, (cg+1)*KC) of all m-subs
                    chunks = []
                    for s in range(N_SUB):
                        t = xc_pool.tile([128, KC * 128], F32, tag="xc")
                        r0 = m0 + s * 128
                        nc.sync.dma_start(
                            t[:], x_d[r0 : r0 + 128, cg * KC * 128 : (cg + 1) * KC * 128]
                        )
                        chunks.append(t)
                    for kk in range(KC):
                        k = cg * KC + kk
                        t_ps = pst.tile([128, MACRO], F32, tag="t")
                        for s in range(N_SUB):
                            nc.tensor.transpose(
                                t_ps[:, s * 128 : (s + 1) * 128],
                                chunks[s][:, kk * 128 : (kk + 1) * 128],
                                ident[:],
                            )
                        xt = xt_pool.tile([128, MACRO], F32R, tag="xt")
                        nc.vector.tensor_copy(xt[:], t_ps[:])
                        nc.tensor.matmul(
                            v_ps[:],
                            a_nat[:, k * RK : (k + 1) * RK],
                            xt[:],
                            start=(k == 0),
                            stop=(k == KT - 1),
                        )
                v_t = vt_pool.tile([RK, MACRO], F32R, tag="vt")
                nc.vector.tensor_copy(v_t[:], v_ps[:])

                for s in range(N_SUB):
                    o_sb = out_pool.tile([128, DL], F32, tag="osb")
                    for d in range(ND):
                        o_ps = pso.tile([128, 1024], F32, tag="o")
                        for h in range(2):
                            nc.tensor.matmul(
                                o_ps[:, h * 512 : (h + 1) * 512],
                                v_t[:, s * 128 : (s + 1) * 128],
                                cat_t[:, (2 * d + h) * 512 : (2 * d + h + 1) * 512],
                                start=True,
                                stop=True,
                            )
                        nc.vector.tensor_copy(
                            o_sb[:, d * 1024 : (d + 1) * 1024], o_ps[:]
                        )
                    r0 = m0 + s * 128
                    nc.scalar.dma_start(o_d[r0 : r0 + 128, :], o_sb[:])

    _split_multi_waits(nc)
    return nc


_NC_CACHE = None


def _get_nc() -> bass.Bass:
    global _NC_CACHE
    if _NC_CACHE is None:
        _NC_CACHE = build_nc()
    return _NC_CACHE


def kernel(x, A, B, W_small) -> np.ndarray:
    x = np.ascontiguousarray(np.asarray(x, dtype=np.float32))
    A = np.ascontiguousarray(np.asarray(A, dtype=np.float32))
    B = np.ascontiguousarray(np.asarray(B, dtype=np.float32))
    W_small = np.ascontiguousarray(np.asarray(W_small, dtype=np.float32))

    xf = x.reshape(M_TOT, DL)
    in_maps = [
        {
            "x": xf[c * M_CORE : (c + 1) * M_CORE],
            "A": A,
            "B": B,
            "W_small": W_small,
        }
        for c in range(N_CORES)
    ]
    nc = _get_nc()
    res = bass_utils.run_bass_kernel_spmd(nc, in_maps, core_ids=list(range(N_CORES)))
    out = np.concatenate([res.results[c]["out"] for c in range(N_CORES)], axis=0)
    return out.reshape(BB, SS, DL)


# revision 6
# speedup vs baseline: 1.2092x; 1.0895x over previous
"""Trainium2 Bass kernel for nn_ProjectedLinear.

Reference computation:
    out = x @ A @ B @ W_small.T @ B.T @ A.T        x:[4,2048,4096]

Algebraic restructuring: C = B @ W_small.T @ B.T is only [64,64], so
    out = (x @ A) @ (C @ A.T)
which cuts the FLOPs to two thin matmuls per token (contract 4096 -> 64,
then 64 -> 4096) and never touches W_small in the main loop.

Distribution: data-parallel over the 8 NeuronCores — each core gets
1024 rows of x (flattened [8192, 4096]) and produces 1024 rows of out.
A/B/W_small are replicated; every core redundantly computes C and
CA^T = C @ A^T on device (small PE ops, hidden under the x DMA).

Per-core dataflow (f32r = tf32-like reduced-precision fp32 matmul mode,
1 cycle/row on the PE vs 4 for exact fp32):
  - x streams in as [128, 512] chunks on the SP HWDGE ring,
  - PE-transposes x k-slices (exact fp32),
  - v^T[64,512] += A[k].T @ x^T[k]     accumulated in PSUM over 32 k-tiles
    (stationary = natural A k-tiles straight from DRAM),
  - out[128,1024] = (v^T slice).T @ CA^T slice   per (m-sub, d) pair,
  - PSUM -> SBUF copies on the DVE; weights + 2 MiB output stores ride
    the ACT HWDGE ring so they never queue behind x loads.
"""

from contextlib import ExitStack

import numpy as np

import concourse.bass as bass
import concourse.mybir as mybir
import concourse.tile as tile
from concourse import bass_utils
from concourse.masks import make_identity

F32 = mybir.dt.float32
F32R = mybir.dt.float32r

N_CORES = 8
BB, SS, DL, RK, DS = 4, 2048, 4096, 64, 768
M_TOT = BB * SS              # 8192 rows
M_CORE = M_TOT // N_CORES    # 1024 rows per core
MACRO = 512                  # rows per macro-tile (mm1 moving dim)
N_MACRO = M_CORE // MACRO    # 2
N_SUB = MACRO // 128         # 4 m-subtiles per macro
KT = DL // 128               # 32 k-tiles
KC = 4                       # k-tiles per x chunk ([128, 512] chunks)
ND = DL // 1024              # 4 d-pairs of 1024 for the output matmul

_wsplit_counter = [0]


def _split_multi_waits(nc) -> int:
    """This container's walrus build rejects >1 sync-wait per instruction
    (one wait slot in the TRN instruction encoding). Split any
    multi-wait instruction into single-wait NoOps on the same engine
    queue directly before it — queue order makes this equivalent."""
    n_split = 0
    for fn in nc.m.functions:
        for blk in fn.blocks:
            out = []
            for inst in blk.instructions:
                si = inst.sync_info
                if si is not None and si.on_wait and len(si.on_wait) > 1:
                    waits = list(si.on_wait)
                    for w in waits[:-1]:
                        _wsplit_counter[0] += 1
                        out.append(
                            mybir.InstNoOp(
                                name=f"wsplit-{_wsplit_counter[0]}",
                                engine=inst.engine,
                                sync_info=mybir.SyncInfo(on_wait=[w], on_update=[]),
                            )
                        )
                    si.on_wait = [waits[-1]]
                    n_split += 1
                out.append(inst)
            if len(out) != len(blk.instructions):
                blk.instructions[:] = out
    return n_split


def build_nc() -> bass.Bass:
    nc = bass.Bass("TRN2", target_bir_lowering=False, debug=False, num_devices=1)

    x_d = nc.dram_tensor("x", [M_CORE, DL], F32, kind="ExternalInput").ap()
    a_d = nc.dram_tensor("A", [DL, RK], F32, kind="ExternalInput").ap()
    b_d = nc.dram_tensor("B", [RK, DS], F32, kind="ExternalInput").ap()
    w_d = nc.dram_tensor("W_small", [DS, DS], F32, kind="ExternalInput").ap()
    o_d = nc.dram_tensor("out", [M_CORE, DL], F32, kind="ExternalOutput").ap()

    with tile.TileContext(nc) as tc, ExitStack() as ctx:
        const = ctx.enter_context(tc.tile_pool(name="const", bufs=1))
        persist = ctx.enter_context(tc.tile_pool(name="persist", bufs=1))

        ident = const.tile([128, 128], F32)
        make_identity(nc, ident[:])

        # Persistent weights: A natural (f32r, mm1 stationary), both as
        # [128, 32*64] (column k holds A[k*128:(k+1)*128, :]), and
        # CA^T = C @ A^T as [64, 4096] (f32r, mm2 moving operand).
        a_nat = persist.tile([128, KT * RK], F32R)
        cat_t = persist.tile([RK, DL], F32R)
        nc.scalar.dma_start(
            a_nat[:].rearrange("p (k r) -> p k r", k=KT),
            a_d.rearrange("(k p) r -> p k r", p=128).bitcast(F32R),
        )

        # ------------- setup: C = B @ W^T @ B^T, A^T, CA^T ------------------
        with (
            tc.tile_pool(name="setup", bufs=1) as sp,
            tc.tile_pool(name="ps_setup", bufs=1, space="PSUM") as pss,
        ):
            NJ = DS // 128  # 6 tiles over the 768 dim
            b_f32 = sp.tile([RK, DS], F32)
            nc.scalar.dma_start(b_f32[:], b_d[:])
            w_tiles = []
            for j in range(NJ):
                wt = sp.tile([128, DS], F32R, tag=f"wt{j}")
                nc.scalar.dma_start(
                    wt[:], w_d[j * 128 : (j + 1) * 128, :].bitcast(F32R)
                )
                w_tiles.append(wt)

            # B^T tiles [128, 64] via PE transpose (exact f32, cast on copy)
            bt_tiles = []
            for j in range(NJ):
                ps = pss.tile([128, RK], F32, tag="ps_bt")
                nc.tensor.transpose(
                    ps[:], b_f32[:, j * 128 : (j + 1) * 128], ident[0:RK, 0:RK]
                )
                bt = sp.tile([128, RK], F32R, tag=f"bt{j}")
                nc.scalar.copy(bt[:], ps[:])
                bt_tiles.append(bt)

            # D[c] = sum_j W[j,c-slice].T @ B^T[j]   -> [128, 64] per c-tile
            d_tiles = []
            for c in range(NJ):
                ps = pss.tile([128, RK], F32, tag="ps_d")
                for j in range(NJ):
                    nc.tensor.matmul(
                        ps[:],
                        w_tiles[j][:, c * 128 : (c + 1) * 128],
                        bt_tiles[j][:],
                        start=(j == 0),
                        stop=(j == NJ - 1),
                    )
                dt = sp.tile([128, RK], F32R, tag=f"d{c}")
                nc.scalar.copy(dt[:], ps[:])
                d_tiles.append(dt)

            # C = sum_c B^T[c].T @ D[c]  -> [64, 64], keep C^T for CA^T
            ps_c = pss.tile([RK, RK], F32, tag="ps_c")
            for c in range(NJ):
                nc.tensor.matmul(
                    ps_c[:],
                    bt_tiles[c][:],
                    d_tiles[c][:],
                    start=(c == 0),
                    stop=(c == NJ - 1),
                )
            c_f32 = sp.tile([RK, RK], F32)
            nc.scalar.copy(c_f32[:], ps_c[:])
            ps_ct = pss.tile([RK, RK], F32, tag="ps_ct")
            nc.tensor.transpose(ps_ct[:], c_f32[:], ident[0:RK, 0:RK])
            ct_t = sp.tile([RK, RK], F32R)
            nc.scalar.copy(ct_t[:], ps_ct[:])

            # A^T via PE transposes, 4 k-tiles per PSUM bank
            at_t = sp.tile([RK, DL], F32R)
            for g in range(KT // 4):
                ps = pss.tile([RK, 512], F32, tag="ps_at")
                for kk in range(4):
                    k = g * 4 + kk
                    nc.tensor.transpose(
                        ps[:, kk * 128 : (kk + 1) * 128],
                        a_nat[:, k * RK : (k + 1) * RK].bitcast(F32),
                        ident[:],
                    )
                nc.scalar.copy(at_t[:, g * 512 : (g + 1) * 512], ps[:])

            # CA^T = C @ A^T = (C^T).T @ A^T  -> [64, 4096]
            for g in range(DL // 512):
                ps = pss.tile([RK, 512], F32, tag="ps_cat")
                nc.tensor.matmul(
                    ps[:],
                    ct_t[:],
                    at_t[:, g * 512 : (g + 1) * 512],
                    start=True,
                    stop=True,
                )
                nc.scalar.copy(cat_t[:, g * 512 : (g + 1) * 512], ps[:])

        # ---------------- main loop ----------------------------------------
        with (
            tc.tile_pool(name="xc", bufs=20) as xc_pool,
            tc.tile_pool(name="xt", bufs=8) as xt_pool,
            tc.tile_pool(name="vt", bufs=2) as vt_pool,
            tc.tile_pool(name="osb", bufs=4) as out_pool,
            tc.tile_pool(name="ps_t", bufs=2, space="PSUM") as pst,
            tc.tile_pool(name="ps_v", bufs=2, space="PSUM") as psv,
            tc.tile_pool(name="ps_o", bufs=2, space="PSUM") as pso,
        ):
            for mb in range(N_MACRO):
                m0 = mb * MACRO
                v_ps = psv.tile([RK, MACRO], F32, tag="v")
                for cg in range(KT // KC):
                    # x chunks for k-tiles [cg*KC, (cg+1)*KC) of all m-subs
                    chunks = []
                    for s in range(N_SUB):
                        t = xc_pool.tile([128, KC * 128], F32, tag="xc")
                        r0 = m0 + s * 128
                        nc.sync.dma_start(
                            t[:], x_d[r0 : r0 + 128, cg * KC * 128 : (cg + 1) * KC * 128]
                        )
                        chunks.append(t)
                    for kk in range(KC):
                        k = cg * KC + kk
                        t_ps = pst.tile([128, MACRO], F32, tag="t")
                        for s in range(N_SUB):
                            nc.tensor.transpose(
                                t_ps[:, s * 128 : (s + 1) * 128],
                                chunks[s][:, kk * 128 : (kk + 1) * 128],
                                ident[:],
                            )
                        xt = xt_pool.tile([128, MACRO], F32R, tag="xt")
                        nc.vector.tensor_copy(xt[:], t_ps[:])
                        nc.tensor.matmul(
                            v_ps[:],
                            a_nat[:, k * RK : (k + 1) * RK],
                            xt[:],
                            start=(k == 0),
                            stop=(k == KT - 1),
                        )
                v_t = vt_pool.tile([RK, MACRO], F32R, tag="vt")
                nc.vector.tensor_copy(v_t[:], v_ps[:])

                for s in range(N_SUB):
                    o_sb = out_pool.tile([128, DL], F32, tag="osb")
                    for d in range(ND):
                        o_ps = pso.tile([128, 1024], F32, tag="o")
                        for h in range(2):
                            nc.tensor.matmul(
                                o_ps[:, h * 512 : (h + 1) * 512],
                                v_t[:, s * 128 : (s + 1) * 128],
                                cat_t[:, (2 * d + h) * 512 : (2 * d + h + 1) * 512],
                                start=True,
                                stop=True,
                            )
                        if d % 2 == 0:
                            nc.vector.tensor_copy(
                                o_sb[:, d * 1024 : (d + 1) * 1024], o_ps[:]
                            )
                        else:
                            nc.scalar.copy(
                                o_sb[:, d * 1024 : (d + 1) * 1024], o_ps[:]
                            )
                    r0 = m0 + s * 128
                    nc.scalar.dma_start(o_d[r0 : r0 + 128, :], o_sb[:])

    _split_multi_waits(nc)
    return nc


_NC_CACHE = None


def _get_nc() -> bass.Bass:
    global _NC_CACHE
    if _NC_CACHE is None:
        _NC_CACHE = build_nc()
    return _NC_CACHE


def kernel(x, A, B, W_small) -> np.ndarray:
    x = np.ascontiguousarray(np.asarray(x, dtype=np.float32))
    A = np.ascontiguousarray(np.asarray(A, dtype=np.float32))
    B = np.ascontiguousarray(np.asarray(B, dtype=np.float32))
    W_small = np.ascontiguousarray(np.asarray(W_small, dtype=np.float32))

    xf = x.reshape(M_TOT, DL)
    in_maps = [
        {
            "x": xf[c * M_CORE : (c + 1) * M_CORE],
            "A": A,
            "B": B,
            "W_small": W_small,
        }
        for c in range(N_CORES)
    ]
    nc = _get_nc()
    res = bass_utils.run_bass_kernel_spmd(nc, in_maps, core_ids=list(range(N_CORES)))
    out = np.concatenate([res.results[c]["out"] for c in range(N_CORES)], axis=0)
    return out.reshape(BB, SS, DL)
